# revision 9
# baseline (speedup 1.0000x reference)
"""Soft k-means EM step (HGNN ClusterNet) on 8 Trainium2 NeuronCores.

Row dimension N=500000 sharded 8 ways (62500 rows/core). Single NEFF launch:
  pass A: stream rows, normalize, build transposed-resident copy, dist+softmax,
          accumulate [17,65] cluster stats in PSUM
  AllReduce the [16,65] partials across cores, mu1 = cluster_mean / cluster_r
  pass B: dist2 = d @ mu1T from the transposed-resident data, softmax, write
          r/dist outputs.
"""
import os
import sys

sys.path.insert(0, "/opt/trn_rl_repo")

import numpy as np

N_CORES = 8
N, D, K = 500000, 64, 16
NC_ROWS = N // N_CORES          # 62500
P = 128
FULL_TILES = NC_ROWS // P       # 488
TAIL = NC_ROWS - FULL_TILES * P  # 36
T_SUP = 8                        # tiles per supertile (softmax batch)
N_SUP = FULL_TILES // T_SUP      # 61
assert N_SUP * T_SUP == FULL_TILES
PAIR_COLS = (FULL_TILES + 1 + 1) // 2   # 245 col-blocks in dataT (incl tail)
CLUSTER_TEMP = 5.0

_cache = {}


def _build(num_iter: int):
    ST = int(os.environ.get("KM_STAGE", "7"))
    import concourse.bass as bass
    import concourse.tile as tile
    from concourse import bacc, mybir
    from concourse.masks import make_identity

    F32 = mybir.dt.float32
    nc = bacc.Bacc("TRN2", target_bir_lowering=False, debug=False,
                   enable_asserts=False, num_devices=N_CORES)

    xin = nc.dram_tensor("xin", [NC_ROWS, D], F32, kind="ExternalInput")
    mu0T_in = nc.dram_tensor("mu0T", [D, K], F32, kind="ExternalInput")
    mu_out = nc.dram_tensor("mu_out", [K, D], F32, kind="ExternalOutput")
    r_out = nc.dram_tensor("r_out", [NC_ROWS, K], F32, kind="ExternalOutput")
    dist_out = nc.dram_tensor("dist_out", [NC_ROWS, K], F32, kind="ExternalOutput")

    cc_ins = [nc.dram_tensor(f"cc_in{i}", [K, D + 1], F32) for i in range(num_iter)]
    muT_scr = [nc.dram_tensor(f"muT_scr{i}", [D, K], F32) for i in range(num_iter)]
    cc_outs = [nc.dram_tensor(f"cc_out{i}", [K, D + 1], F32, addr_space="Shared")
               for i in range(num_iter)]

    with tile.TileContext(nc) as tc:
        import contextlib
        with contextlib.ExitStack() as ctx:
            const = ctx.enter_context(tc.tile_pool(name="const", bufs=1))
            big = ctx.enter_context(tc.tile_pool(name="big", bufs=1))
            stage = ctx.enter_context(tc.tile_pool(name="stage", bufs=4))
            rpool = ctx.enter_context(tc.tile_pool(name="rpool", bufs=3))
            spool = ctx.enter_context(tc.tile_pool(name="spool", bufs=3))
            opool = ctx.enter_context(tc.tile_pool(name="opool", bufs=4))
            psum = ctx.enter_context(tc.tile_pool(name="psum", bufs=2, space="PSUM"))
            tpsum = ctx.enter_context(tc.tile_pool(name="tpsum", bufs=2, space="PSUM"))
            apsum = ctx.enter_context(tc.tile_pool(name="apsum", bufs=1, space="PSUM"))

            ident = const.tile([P, P], F32)
            make_identity(nc, ident)
            ones = const.tile([P, 1], F32)
            nc.vector.memset(ones, 1.0)
            mu0T = const.tile([P, 2 * K], F32)
            nc.vector.memset(mu0T, 0.0)
            nc.sync.dma_start(out=mu0T[0:D, 0:K], in_=mu0T_in[:, :])
            nc.sync.dma_start(out=mu0T[D:P, K:2 * K], in_=mu0T_in[:, :])
            # transposed-resident normalized data: tile t at partitions
            # (t%2)*64, col block t//2
            dataT = big.tile([P, PAIR_COLS, P], F32)
            muT_cur = mu0T


            def softmax_tiles(dist_ps, nt, r_ext, it, sup_tag):
                """dist_ps: psum [rows, nt*K]; writes r into r_ext[:, :, 0:K]."""
                rows = dist_ps.shape[0]
                ex = spool.tile([P, T_SUP, K], F32, tag=f"exp{sup_tag}")
                nc.scalar.activation(
                    out=ex[:rows, :nt, :],
                    in_=dist_ps.rearrange("p (t k) -> p t k", k=K),
                    func=mybir.ActivationFunctionType.Exp,
                    scale=CLUSTER_TEMP,
                )
                rs = spool.tile([P, T_SUP], F32, tag=f"rs{sup_tag}")
                nc.vector.reduce_sum(rs[:rows, :nt], ex[:rows, :nt, :],
                                     axis=mybir.AxisListType.X)
                nc.vector.reciprocal(rs[:rows, :nt], rs[:rows, :nt])
                nc.vector.tensor_mul(
                    r_ext[:rows, :nt, 0:K], ex[:rows, :nt, :],
                    rs[:rows, :nt].rearrange("p (t o) -> p t o", o=1)
                      .to_broadcast((rows, nt, K)),
                )
                return ex

            # ---------------- pass A (once per EM iteration) ----------------
            for it in range(num_iter):
                stats_ps = apsum.tile([K + 1, D], F32, tag="sA")
                statsr_ps = apsum.tile([K + 1, 1], F32, tag="sR")
                first_pass = it == 0
                for s in range(N_SUP + 1):
                    nt = T_SUP if s < N_SUP else 1
                    rows = P if s < N_SUP else TAIL
                    n0 = s * T_SUP * P
                    if True:
                        dst = stage.tile([P, T_SUP, D], F32, tag="dstage")
                        if s < N_SUP:
                            nc.sync.dma_start(
                                out=dst[:, :, :],
                                in_=xin[n0:n0 + nt * P, :].rearrange(
                                    "(t p) d -> p t d", p=P),
                            )
                        else:
                            nc.sync.dma_start(out=dst[:rows, 0, :],
                                              in_=xin[n0:n0 + rows, :])
                        # normalize rows
                        sq = spool.tile([P, T_SUP, D], F32, tag="sq")
                        nc.vector.tensor_mul(sq[:rows, :nt, :],
                                             dst[:rows, :nt, :],
                                             dst[:rows, :nt, :])
                        sums = spool.tile([P, T_SUP], F32, tag="sums")
                        nc.vector.reduce_sum(sums[:rows, :nt], sq[:rows, :nt, :],
                                             axis=mybir.AxisListType.X)
                        nc.scalar.activation(out=sums[:rows, :nt],
                                             in_=sums[:rows, :nt],
                                             func=mybir.ActivationFunctionType.Sqrt)
                        nc.vector.reciprocal(sums[:rows, :nt], sums[:rows, :nt])
                        nc.vector.tensor_mul(
                            dst[:rows, :nt, :], dst[:rows, :nt, :],
                            sums[:rows, :nt].rearrange("p (t o) -> p t o", o=1)
                                .to_broadcast((rows, nt, D)),
                        )
                        # paired transposes into dataT (first iteration only)
                        for j in range(0, nt, 2) if (first_pass and ST >= 2) else []:
                            t_glob = s * T_SUP + j
                            cb = t_glob // 2
                            if j + 1 < nt:
                                tp = tpsum.tile([P, P], F32, tag="tp")
                                nc.tensor.transpose(
                                    tp, dst[:, j:j + 2, :], ident)
                                if cb % 2 == 0:
                                    nc.vector.tensor_copy(dataT[:, cb, :], tp)
                                else:
                                    nc.scalar.copy(out=dataT[:, cb, :], in_=tp)
                            else:  # tail single tile
                                tp = tpsum.tile([P, P], F32, tag="tp")
                                nc.tensor.transpose(
                                    tp[0:D, 0:rows], dst[:rows, j, :],
                                    ident[:rows, :rows])
                                nc.vector.tensor_copy(
                                    dataT[0:D, cb, 0:rows], tp[0:D, 0:rows])
                    # dist for each tile in supertile
                    if ST < 3:
                        continue
                    dist_ps = psum.tile([P, T_SUP * K], F32, tag="dist")
                    for j2 in range(0, nt, 2):
                        t_glob = s * T_SUP + j2
                        cb = t_glob // 2
                        if j2 + 1 < nt:
                            nc.tensor.matmul(
                                dist_ps[:, j2 * K:(j2 + 2) * K],
                                dataT[:, cb, :], muT_cur,
                                start=True, stop=True)
                        else:
                            nc.tensor.matmul(
                                dist_ps[:TAIL, j2 * K:(j2 + 1) * K],
                                dataT[0:D, cb, 0:TAIL], muT_cur[0:D, 0:K],
                                start=True, stop=True)
                    if ST < 4:
                        continue
                    r_ext = rpool.tile([P, T_SUP, K + 1], F32, tag="rext")
                    nc.vector.memset(r_ext[:, :, K:K + 1], 1.0)
                    softmax_tiles(dist_ps[:rows, 0:nt * K], nt, r_ext, it, "A")
                    if ST < 5:
                        continue
                    for j in range(nt):
                        t_glob = s * T_SUP + j
                        nrows = P if t_glob < FULL_TILES else TAIL
                        st = t_glob == 0
                        sp = t_glob == FULL_TILES
                        nc.tensor.matmul(
                            stats_ps, r_ext[:nrows, j, :],
                            dst[:nrows, j, :], start=st, stop=sp)
                        nc.tensor.matmul(
                            statsr_ps, r_ext[:nrows, j, :],
                            ones[:nrows], start=st, stop=sp)
                # stats -> sbuf -> AllReduce -> mu1
                if ST < 6:
                    continue
                stats_sb = const.tile([K, D + 1], F32, tag=f"stats{it}")
                nc.vector.tensor_copy(stats_sb[:, 0:D], stats_ps[0:K, :])
                nc.vector.tensor_copy(stats_sb[:, D:D + 1], statsr_ps[0:K, :])
                nc.sync.dma_start(out=cc_ins[it][:, :], in_=stats_sb)
                nc.gpsimd.collective_compute(
                    "AllReduce", mybir.AluOpType.add,
                    replica_groups=[list(range(N_CORES))],
                    ins=[cc_ins[it].ap().opt()], outs=[cc_outs[it].ap().opt()],
                )
                stats_g = const.tile([K, D + 1], F32, tag=f"statsg{it}")
                nc.sync.dma_start(out=stats_g, in_=cc_outs[it][:, :])
                nc.vector.reciprocal(stats_g[:, D:D + 1], stats_g[:, D:D + 1])
                mu1 = const.tile([K, D], F32, tag=f"mu1_{it}")
                nc.vector.tensor_scalar_mul(mu1, in0=stats_g[:, 0:D],
                                            scalar1=stats_g[:, D:D + 1])
                mu1T_ps = tpsum.tile([D, K], F32, tag="tp")
                nc.tensor.transpose(mu1T_ps, mu1, ident[0:K, 0:K])
                mu1T = const.tile([P, 2 * K], F32, tag=f"mu1T_{it}")
                nc.vector.memset(mu1T, 0.0)
                nc.vector.tensor_copy(mu1T[0:D, 0:K], mu1T_ps)
                nc.sync.dma_start(out=muT_scr[it][:, :], in_=mu1T[0:D, 0:K])
                nc.sync.dma_start(out=mu1T[D:P, K:2 * K], in_=muT_scr[it][:, :])
                muT_cur = mu1T
                if it == num_iter - 1:
                    nc.sync.dma_start(out=mu_out[:, :], in_=mu1)

            if num_iter == 0:
                # mu stays mu0 (normalized): emit it (host supplies mu0T)
                mu0 = const.tile([K, D], F32, tag="mu0row")
                m0ps = tpsum.tile([K, D], F32, tag="tp")
                nc.tensor.transpose(m0ps, mu0T[0:D, 0:K], ident[0:D, 0:D])
                nc.vector.tensor_copy(mu0, m0ps)
                nc.sync.dma_start(out=mu_out[:, :], in_=mu0)

            # ---------------- pass B: final dist / r ----------------
            for s in (range(N_SUP + 1) if ST >= 7 else []):
                nt = T_SUP if s < N_SUP else 1
                rows = P if s < N_SUP else TAIL
                n0 = s * T_SUP * P
                dist_ps = psum.tile([P, T_SUP * K], F32, tag="dist")
                for j2 in range(0, nt, 2):
                    t_glob = s * T_SUP + j2
                    cb = t_glob // 2
                    if j2 + 1 < nt:
                        nc.tensor.matmul(
                            dist_ps[:, j2 * K:(j2 + 2) * K],
                            dataT[:, cb, :], muT_cur,
                            start=True, stop=True)
                    else:
                        nc.tensor.matmul(
                            dist_ps[:TAIL, j2 * K:(j2 + 1) * K],
                            dataT[0:D, cb, 0:TAIL], muT_cur[0:D, 0:K],
                            start=True, stop=True)
                dist_sb = opool.tile([P, T_SUP, K], F32, tag="distsb")
                nc.scalar.copy(
                    out=dist_sb[:rows, :nt, :],
                    in_=dist_ps[:rows, 0:nt * K].rearrange("p (t k) -> p t k", k=K))
                r_sb = opool.tile([P, T_SUP, K + 1], F32, tag="rsb")
                softmax_tiles(dist_ps[:rows, 0:nt * K], nt, r_sb, 99, "B")
                if s < N_SUP:
                    nc.sync.dma_start(
                        out=dist_out[n0:n0 + nt * P, :].rearrange(
                            "(t p) k -> p t k", p=P),
                        in_=dist_sb[:, :nt, :])
                    nc.sync.dma_start(
                        out=r_out[n0:n0 + nt * P, :].rearrange(
                            "(t p) k -> p t k", p=P),
                        in_=r_sb[:, :nt, 0:K])
                else:
                    nc.sync.dma_start(out=dist_out[n0:n0 + rows, :],
                                      in_=dist_sb[:rows, 0, :])
                    nc.sync.dma_start(out=r_out[n0:n0 + rows, :],
                                      in_=r_sb[:rows, 0, 0:K])
    nc.compile()
    return nc


def _get(num_iter: int):
    if num_iter not in _cache:
        _cache[num_iter] = _build(num_iter)
    return _cache[num_iter]


def kernel(data, mu_init, num_iter):
    from concourse.bass_utils import run_bass_kernel_spmd

    data = np.asarray(data, dtype=np.float32)
    mu_init = np.asarray(mu_init, dtype=np.float32)
    ni = int(num_iter)
    nc = _get(ni)

    mu0 = mu_init / np.linalg.norm(mu_init, axis=1, keepdims=True)
    mu0T = np.ascontiguousarray(mu0.T)
    in_maps = [
        {"xin": np.ascontiguousarray(data[c * NC_ROWS:(c + 1) * NC_ROWS]),
         "mu0T": mu0T}
        for c in range(N_CORES)
    ]
    res = run_bass_kernel_spmd(nc, in_maps, core_ids=list(range(N_CORES)))
    mu = res.results[0]["mu_out"]
    r = np.concatenate([res.results[c]["r_out"] for c in range(N_CORES)], axis=0)
    dist = np.concatenate([res.results[c]["dist_out"] for c in range(N_CORES)],
                          axis=0)
    return mu, r, dist


# revision 10
# speedup vs baseline: 1.0121x; 1.0121x over previous
"""Soft k-means EM step (HGNN ClusterNet) on 8 Trainium2 NeuronCores.

Row dimension N=500000 sharded 8 ways (62500 rows/core). Single NEFF launch:
  pass A: stream rows, normalize, build transposed-resident copy, dist+softmax,
          accumulate [17,65] cluster stats in PSUM
  AllReduce the [16,65] partials across cores, mu1 = cluster_mean / cluster_r
  pass B: dist2 = d @ mu1T from the transposed-resident data, softmax, write
          r/dist outputs.
"""
import os
import sys

sys.path.insert(0, "/opt/trn_rl_repo")

import numpy as np

N_CORES = 8
N, D, K = 500000, 64, 16
NC_ROWS = N // N_CORES          # 62500
P = 128
FULL_TILES = NC_ROWS // P       # 488
TAIL = NC_ROWS - FULL_TILES * P  # 36
T_SUP = 8                        # tiles per supertile (softmax batch)
N_SUP = FULL_TILES // T_SUP      # 61
assert N_SUP * T_SUP == FULL_TILES
PAIR_COLS = (FULL_TILES + 1 + 1) // 2   # 245 col-blocks in dataT (incl tail)
CLUSTER_TEMP = 5.0

_cache = {}


def _build(num_iter: int):
    ST = int(os.environ.get("KM_STAGE", "7"))
    import concourse.bass as bass
    import concourse.tile as tile
    from concourse import bacc, mybir
    from concourse.masks import make_identity

    F32 = mybir.dt.float32
    nc = bacc.Bacc("TRN2", target_bir_lowering=False, debug=False,
                   enable_asserts=False, num_devices=N_CORES)

    xin = nc.dram_tensor("xin", [NC_ROWS, D], F32, kind="ExternalInput")
    mu0T_in = nc.dram_tensor("mu0T", [D, K], F32, kind="ExternalInput")
    mu_out = nc.dram_tensor("mu_out", [K, D], F32, kind="ExternalOutput")
    r_out = nc.dram_tensor("r_out", [NC_ROWS, K], F32, kind="ExternalOutput")
    dist_out = nc.dram_tensor("dist_out", [NC_ROWS, K], F32, kind="ExternalOutput")

    cc_ins = [nc.dram_tensor(f"cc_in{i}", [K, D + 1], F32) for i in range(num_iter)]
    muT_scr = [nc.dram_tensor(f"muT_scr{i}", [D, K], F32) for i in range(num_iter)]
    cc_outs = [nc.dram_tensor(f"cc_out{i}", [K, D + 1], F32, addr_space="Shared")
               for i in range(num_iter)]

    with tile.TileContext(nc) as tc:
        import contextlib
        with contextlib.ExitStack() as ctx:
            const = ctx.enter_context(tc.tile_pool(name="const", bufs=1))
            big = ctx.enter_context(tc.tile_pool(name="big", bufs=1))
            stage = ctx.enter_context(tc.tile_pool(name="stage", bufs=6))
            rpool = ctx.enter_context(tc.tile_pool(name="rpool", bufs=4))
            spool = ctx.enter_context(tc.tile_pool(name="spool", bufs=3))
            opool = ctx.enter_context(tc.tile_pool(name="opool", bufs=6))
            psum = ctx.enter_context(tc.tile_pool(name="psum", bufs=3, space="PSUM"))
            tpsum = ctx.enter_context(tc.tile_pool(name="tpsum", bufs=2, space="PSUM"))
            apsum = ctx.enter_context(tc.tile_pool(name="apsum", bufs=1, space="PSUM"))

            ident = const.tile([P, P], F32)
            make_identity(nc, ident)
            ones = const.tile([P, 1], F32)
            nc.vector.memset(ones, 1.0)
            mu0T = const.tile([P, 2 * K], F32)
            nc.vector.memset(mu0T, 0.0)
            nc.sync.dma_start(out=mu0T[0:D, 0:K], in_=mu0T_in[:, :])
            nc.sync.dma_start(out=mu0T[D:P, K:2 * K], in_=mu0T_in[:, :])
            # transposed-resident normalized data: tile t at partitions
            # (t%2)*64, col block t//2
            dataT = big.tile([P, PAIR_COLS, P], F32)
            muT_cur = mu0T


            def softmax_tiles(dist_ps, nt, r_ext, it, sup_tag):
                """dist_ps: psum [rows, nt*K]; writes r into r_ext[:, :, 0:K]."""
                rows = dist_ps.shape[0]
                ex = spool.tile([P, T_SUP, K], F32, tag=f"exp{sup_tag}")
                nc.scalar.activation(
                    out=ex[:rows, :nt, :],
                    in_=dist_ps.rearrange("p (t k) -> p t k", k=K),
                    func=mybir.ActivationFunctionType.Exp,
                    scale=CLUSTER_TEMP,
                )
                rs = spool.tile([P, T_SUP], F32, tag=f"rs{sup_tag}")
                nc.vector.reduce_sum(rs[:rows, :nt], ex[:rows, :nt, :],
                                     axis=mybir.AxisListType.X)
                nc.vector.reciprocal(rs[:rows, :nt], rs[:rows, :nt])
                nc.vector.tensor_mul(
                    r_ext[:rows, :nt, 0:K], ex[:rows, :nt, :],
                    rs[:rows, :nt].rearrange("p (t o) -> p t o", o=1)
                      .to_broadcast((rows, nt, K)),
                )
                return ex

            # ---------------- pass A (once per EM iteration) ----------------
            for it in range(num_iter):
                stats_ps = apsum.tile([K + 1, D], F32, tag="sA")
                statsr_ps = apsum.tile([K + 1, 1], F32, tag="sR")
                first_pass = it == 0
                for s in range(N_SUP + 1):
                    nt = T_SUP if s < N_SUP else 1
                    rows = P if s < N_SUP else TAIL
                    n0 = s * T_SUP * P
                    if True:
                        dst = stage.tile([P, T_SUP, D], F32, tag="dstage")
                        if s < N_SUP:
                            nc.sync.dma_start(
                                out=dst[:, :, :],
                                in_=xin[n0:n0 + nt * P, :].rearrange(
                                    "(t p) d -> p t d", p=P),
                            )
                        else:
                            nc.sync.dma_start(out=dst[:rows, 0, :],
                                              in_=xin[n0:n0 + rows, :])
                        # normalize rows
                        sq = spool.tile([P, T_SUP, D], F32, tag="sq")
                        nc.vector.tensor_mul(sq[:rows, :nt, :],
                                             dst[:rows, :nt, :],
                                             dst[:rows, :nt, :])
                        sums = spool.tile([P, T_SUP], F32, tag="sums")
                        nc.vector.reduce_sum(sums[:rows, :nt], sq[:rows, :nt, :],
                                             axis=mybir.AxisListType.X)
                        nc.scalar.activation(out=sums[:rows, :nt],
                                             in_=sums[:rows, :nt],
                                             func=mybir.ActivationFunctionType.Sqrt)
                        nc.vector.reciprocal(sums[:rows, :nt], sums[:rows, :nt])
                        nc.vector.tensor_mul(
                            dst[:rows, :nt, :], dst[:rows, :nt, :],
                            sums[:rows, :nt].rearrange("p (t o) -> p t o", o=1)
                                .to_broadcast((rows, nt, D)),
                        )
                        # paired transposes into dataT (first iteration only)
                        for j in range(0, nt, 2) if (first_pass and ST >= 2) else []:
                            t_glob = s * T_SUP + j
                            cb = t_glob // 2
                            if j + 1 < nt:
                                tp = tpsum.tile([P, P], F32, tag="tp")
                                nc.tensor.transpose(
                                    tp, dst[:, j:j + 2, :], ident)
                                if cb % 2 == 0:
                                    nc.vector.tensor_copy(dataT[:, cb, :], tp)
                                else:
                                    nc.scalar.copy(out=dataT[:, cb, :], in_=tp)
                            else:  # tail single tile
                                tp = tpsum.tile([P, P], F32, tag="tp")
                                nc.tensor.transpose(
                                    tp[0:D, 0:rows], dst[:rows, j, :],
                                    ident[:rows, :rows])
                                nc.vector.tensor_copy(
                                    dataT[0:D, cb, 0:rows], tp[0:D, 0:rows])
                    # dist for each tile in supertile
                    if ST < 3:
                        continue
                    dist_ps = psum.tile([P, T_SUP * K], F32, tag="dist")
                    for j2 in range(0, nt, 2):
                        t_glob = s * T_SUP + j2
                        cb = t_glob // 2
                        if j2 + 1 < nt:
                            nc.tensor.matmul(
                                dist_ps[:, j2 * K:(j2 + 2) * K],
                                dataT[:, cb, :], muT_cur,
                                start=True, stop=True)
                        else:
                            nc.tensor.matmul(
                                dist_ps[:TAIL, j2 * K:(j2 + 1) * K],
                                dataT[0:D, cb, 0:TAIL], muT_cur[0:D, 0:K],
                                start=True, stop=True)
                    if ST < 4:
                        continue
                    r_ext = rpool.tile([P, T_SUP, K + 1], F32, tag="rext")
                    nc.vector.memset(r_ext[:, :, K:K + 1], 1.0)
                    softmax_tiles(dist_ps[:rows, 0:nt * K], nt, r_ext, it, "A")
                    if ST < 5:
                        continue
                    for j in range(nt):
                        t_glob = s * T_SUP + j
                        nrows = P if t_glob < FULL_TILES else TAIL
                        st = t_glob == 0
                        sp = t_glob == FULL_TILES
                        nc.tensor.matmul(
                            stats_ps, r_ext[:nrows, j, :],
                            dst[:nrows, j, :], start=st, stop=sp)
                        nc.tensor.matmul(
                            statsr_ps, r_ext[:nrows, j, :],
                            ones[:nrows], start=st, stop=sp)
                # stats -> sbuf -> AllReduce -> mu1
                if ST < 6:
                    continue
                stats_sb = const.tile([K, D + 1], F32, tag=f"stats{it}")
                nc.vector.tensor_copy(stats_sb[:, 0:D], stats_ps[0:K, :])
                nc.vector.tensor_copy(stats_sb[:, D:D + 1], statsr_ps[0:K, :])
                nc.sync.dma_start(out=cc_ins[it][:, :], in_=stats_sb)
                nc.gpsimd.collective_compute(
                    "AllReduce", mybir.AluOpType.add,
                    replica_groups=[list(range(N_CORES))],
                    ins=[cc_ins[it].ap().opt()], outs=[cc_outs[it].ap().opt()],
                )
                stats_g = const.tile([K, D + 1], F32, tag=f"statsg{it}")
                nc.sync.dma_start(out=stats_g, in_=cc_outs[it][:, :])
                nc.vector.reciprocal(stats_g[:, D:D + 1], stats_g[:, D:D + 1])
                mu1 = const.tile([K, D], F32, tag=f"mu1_{it}")
                nc.vector.tensor_scalar_mul(mu1, in0=stats_g[:, 0:D],
                                            scalar1=stats_g[:, D:D + 1])
                mu1T_ps = tpsum.tile([D, K], F32, tag="tp")
                nc.tensor.transpose(mu1T_ps, mu1, ident[0:K, 0:K])
                mu1T = const.tile([P, 2 * K], F32, tag=f"mu1T_{it}")
                nc.vector.memset(mu1T, 0.0)
                nc.vector.tensor_copy(mu1T[0:D, 0:K], mu1T_ps)
                nc.sync.dma_start(out=muT_scr[it][:, :], in_=mu1T[0:D, 0:K])
                nc.sync.dma_start(out=mu1T[D:P, K:2 * K], in_=muT_scr[it][:, :])
                muT_cur = mu1T
                if it == num_iter - 1:
                    nc.sync.dma_start(out=mu_out[:, :], in_=mu1)

            if num_iter == 0:
                # mu stays mu0 (normalized): emit it (host supplies mu0T)
                mu0 = const.tile([K, D], F32, tag="mu0row")
                m0ps = tpsum.tile([K, D], F32, tag="tp")
                nc.tensor.transpose(m0ps, mu0T[0:D, 0:K], ident[0:D, 0:D])
                nc.vector.tensor_copy(mu0, m0ps)
                nc.sync.dma_start(out=mu_out[:, :], in_=mu0)

            # ---------------- pass B: final dist / r ----------------
            for s in (range(N_SUP + 1) if ST >= 7 else []):
                nt = T_SUP if s < N_SUP else 1
                rows = P if s < N_SUP else TAIL
                n0 = s * T_SUP * P
                dist_ps = psum.tile([P, T_SUP * K], F32, tag="dist")
                for j2 in range(0, nt, 2):
                    t_glob = s * T_SUP + j2
                    cb = t_glob // 2
                    if j2 + 1 < nt:
                        nc.tensor.matmul(
                            dist_ps[:, j2 * K:(j2 + 2) * K],
                            dataT[:, cb, :], muT_cur,
                            start=True, stop=True)
                    else:
                        nc.tensor.matmul(
                            dist_ps[:TAIL, j2 * K:(j2 + 1) * K],
                            dataT[0:D, cb, 0:TAIL], muT_cur[0:D, 0:K],
                            start=True, stop=True)
                dist_sb = opool.tile([P, T_SUP, K], F32, tag="distsb")
                nc.scalar.copy(
                    out=dist_sb[:rows, :nt, :],
                    in_=dist_ps[:rows, 0:nt * K].rearrange("p (t k) -> p t k", k=K))
                r_sb = opool.tile([P, T_SUP, K + 1], F32, tag="rsb")
                softmax_tiles(dist_ps[:rows, 0:nt * K], nt, r_sb, 99, "B")
                if s < N_SUP:
                    nc.sync.dma_start(
                        out=dist_out[n0:n0 + nt * P, :].rearrange(
                            "(t p) k -> p t k", p=P),
                        in_=dist_sb[:, :nt, :])
                    nc.sync.dma_start(
                        out=r_out[n0:n0 + nt * P, :].rearrange(
                            "(t p) k -> p t k", p=P),
                        in_=r_sb[:, :nt, 0:K])
                else:
                    nc.sync.dma_start(out=dist_out[n0:n0 + rows, :],
                                      in_=dist_sb[:rows, 0, :])
                    nc.sync.dma_start(out=r_out[n0:n0 + rows, :],
                                      in_=r_sb[:rows, 0, 0:K])
    nc.compile()
    return nc


def _get(num_iter: int):
    if num_iter not in _cache:
        _cache[num_iter] = _build(num_iter)
    return _cache[num_iter]


def kernel(data, mu_init, num_iter):
    from concourse.bass_utils import run_bass_kernel_spmd

    data = np.asarray(data, dtype=np.float32)
    mu_init = np.asarray(mu_init, dtype=np.float32)
    ni = int(num_iter)
    nc = _get(ni)

    mu0 = mu_init / np.linalg.norm(mu_init, axis=1, keepdims=True)
    mu0T = np.ascontiguousarray(mu0.T)
    in_maps = [
        {"xin": np.ascontiguousarray(data[c * NC_ROWS:(c + 1) * NC_ROWS]),
         "mu0T": mu0T}
        for c in range(N_CORES)
    ]
    res = run_bass_kernel_spmd(nc, in_maps, core_ids=list(range(N_CORES)))
    mu = res.results[0]["mu_out"]
    r = np.concatenate([res.results[c]["r_out"] for c in range(N_CORES)], axis=0)
    dist = np.concatenate([res.results[c]["dist_out"] for c in range(N_CORES)],
                          axis=0)
    return mu, r, dist
